# Initial kernel scaffold
#
"""Bahdanau additive attention on 8 Trainium2 NeuronCores.

Sharding: data-parallel over batch b (B=32 -> 4 per core); parameters replicated.

Per-core layout: encoder tensors (T=2048, bl=4, N=1024) flattened to rows=(t*4+b)
-> 64 row-tiles of [128, 1024].  Partition p of tile i holds (t = 32*i + p//4,
b = p%4).  Per tile:
  DVE : e_pre = f + dec_rep          (dec_rep[p,:] = dec_feature[p%4,:])
  ACT : e = tanh(e_pre)
  DVE : score_col = sum_n e * v_rep  (fused scalar_tensor_tensor accum)
  ACT : exp_col = exp(score_col)     (softmax max-shift unneeded: |scores| < 30)
  DVE : A = (sel4 * exp_col) * mask_col   [128, 4] one-hot-by-batch rhs
  PE  : content[nchunk, b] += o_tile[:, nchunk].T @ A   (8 matmuls, PSUM accum)
Tail: den[b] = sum_t exp (PE sel4 matmul + DVE reduce), content scaled by 1/den
inside the final linear; hx = tanh(lin_w @ [content; hx_dec] + lin_b) on PE/DVE/ACT.
v_b shifts all logits uniformly -> cancels in softmax -> dropped.
"""
import os
import sys

sys.path.insert(0, "/opt/trn_rl_repo")

import numpy as np

T, B, N = 2048, 32, 1024
NCORES = 8
BL = B // NCORES              # 4 local batches per core
ROWS = T * BL                 # 8192
NT = ROWS // 128              # 64 row tiles
NCH = N // 128                # 8 column chunks of N

LAST_RESULT = None            # BassKernelResults of the most recent run (for test.py)


def _build():
    import concourse.bacc as bacc
    import concourse.tile as tile
    from concourse import mybir
    from contextlib import ExitStack

    dt = mybir.dt
    f32 = dt.float32
    Alu = mybir.AluOpType
    Act = mybir.ActivationFunctionType

    nc = bacc.Bacc("TRN2")
    ef = nc.dram_tensor("ef", [ROWS, N], f32, kind="ExternalInput")
    eo = nc.dram_tensor("eo", [ROWS, N], f32, kind="ExternalInput")
    hxT = nc.dram_tensor("hxT", [N, BL], f32, kind="ExternalInput")      # decoder_hx slice, transposed
    wsT = nc.dram_tensor("wsT", [N, N], f32, kind="ExternalInput")       # Ws_w.T (nin, nout)
    wsb = nc.dram_tensor("wsb", [BL, N], f32, kind="ExternalInput")      # Ws_b tiled 4x
    vw = nc.dram_tensor("vw", [1, N], f32, kind="ExternalInput")
    maskp = nc.dram_tensor("maskp", [128, NT], f32, kind="ExternalInput")
    linT = nc.dram_tensor("linT", [2 * N, N], f32, kind="ExternalInput") # lin_w.T (k, j)
    linb = nc.dram_tensor("linb", [BL, N], f32, kind="ExternalInput")    # lin_b tiled 4x
    sel4 = nc.dram_tensor("sel4", [128, BL], f32, kind="ExternalInput")  # tile(eye(4), (32,1))
    rep4 = nc.dram_tensor("rep4", [BL, 128], f32, kind="ExternalInput")  # sel4.T
    ones128 = nc.dram_tensor("ones128", [1, 128], f32, kind="ExternalInput")
    out = nc.dram_tensor("out", [BL, N], f32, kind="ExternalOutput")

    with tile.TileContext(nc) as tc, ExitStack() as ctx:
        const = ctx.enter_context(tc.tile_pool(name="const", bufs=1))
        wpool = ctx.enter_context(tc.tile_pool(name="weights", bufs=1))
        psum = ctx.enter_context(tc.tile_pool(name="psum", bufs=2, space="PSUM"))
        psc = ctx.enter_context(tc.tile_pool(name="psc", bufs=1, space="PSUM"))
        fpool = ctx.enter_context(tc.tile_pool(name="f", bufs=4))
        opool = ctx.enter_context(tc.tile_pool(name="o", bufs=4))
        e1pool = ctx.enter_context(tc.tile_pool(name="e1", bufs=2))
        e2pool = ctx.enter_context(tc.tile_pool(name="e2", bufs=2))
        spool = ctx.enter_context(tc.tile_pool(name="small", bufs=4))

        # ---- setup: params in ----
        wsT_sb = wpool.tile([128, NCH, N], f32)
        nc.sync.dma_start(out=wsT_sb, in_=wsT[:].rearrange("(c p) n -> p c n", p=128))
        linT_sb = wpool.tile([128, 2 * NCH, N], f32)
        nc.sync.dma_start(out=linT_sb, in_=linT[:].rearrange("(c p) j -> p c j", p=128))
        hxT_sb = const.tile([128, NCH, BL], f32)
        nc.sync.dma_start(out=hxT_sb, in_=hxT[:].rearrange("(c p) b -> p c b", p=128))
        wsb_sb = const.tile([BL, N], f32)
        nc.sync.dma_start(out=wsb_sb, in_=wsb[:])
        vw_sb = const.tile([1, N], f32)
        nc.sync.dma_start(out=vw_sb, in_=vw[:])
        maskp_sb = const.tile([128, NT], f32)
        nc.sync.dma_start(out=maskp_sb, in_=maskp[:])
        sel4_sb = const.tile([128, BL], f32)
        nc.sync.dma_start(out=sel4_sb, in_=sel4[:])
        rep4_sb = const.tile([BL, 128], f32)
        nc.sync.dma_start(out=rep4_sb, in_=rep4[:])
        ones_sb = const.tile([1, 128], f32)
        nc.sync.dma_start(out=ones128[:] if False else ones_sb, in_=ones128[:])
        linb_sb = const.tile([BL, N], f32)
        nc.sync.dma_start(out=linb_sb, in_=linb[:])

        # ---- dec_feature = decoder_hx @ Ws_w.T + Ws_b, then replicate to 128 rows ----
        dec_ps = psum.tile([BL, N], f32, tag="aux")
        for h in range(2):
            for ci in range(NCH):
                nc.tensor.matmul(
                    dec_ps[:, 512 * h:512 * (h + 1)],
                    lhsT=hxT_sb[:, ci, :],
                    rhs=wsT_sb[:, ci, 512 * h:512 * (h + 1)],
                    start=(ci == 0), stop=(ci == NCH - 1))
        dec_sb = const.tile([BL, N], f32)
        nc.vector.tensor_add(dec_sb, dec_ps, wsb_sb)
        rep_ps = psum.tile([128, N], f32, tag="aux")
        for h in range(2):
            nc.tensor.matmul(rep_ps[:, 512 * h:512 * (h + 1)],
                             lhsT=rep4_sb, rhs=dec_sb[:, 512 * h:512 * (h + 1)],
                             start=True, stop=True)
        dec_rep = const.tile([128, N], f32)
        nc.vector.tensor_copy(dec_rep, rep_ps)

        # ---- v_w replicated to 128 rows via PE ----
        vrep_ps = psum.tile([128, N], f32, tag="aux")
        for h in range(2):
            nc.tensor.matmul(vrep_ps[:, 512 * h:512 * (h + 1)],
                             lhsT=ones_sb, rhs=vw_sb[:, 512 * h:512 * (h + 1)],
                             start=True, stop=True)
        v_rep = const.tile([128, N], f32)
        nc.vector.tensor_copy(v_rep, vrep_ps)

        exp_all = const.tile([128, NT], f32)
        content_ps = psc.tile([128, NCH * BL], f32)   # [n%128, chunk*4+b]

        ef_ap, eo_ap = ef[:], eo[:]
        # ---- main streaming loop ----
        for i in range(NT):
            r0 = 128 * i
            f_t = fpool.tile([128, N], f32)
            nc.sync.dma_start(out=f_t, in_=ef_ap[r0:r0 + 128, :])
            ep = e1pool.tile([128, N], f32)
            nc.vector.tensor_add(ep, f_t, dec_rep)
            e_t = e2pool.tile([128, N], f32)
            nc.scalar.activation(e_t, ep, Act.Tanh)
            dummy = spool.tile([128, 1], f32, tag="dummy")
            nc.vector.scalar_tensor_tensor(
                out=dummy.broadcast_to(e_t.shape), in0=e_t, scalar=1.0, in1=v_rep,
                op0=Alu.mult, op1=Alu.mult, accum_out=exp_all[:, i:i + 1])
            nc.scalar.activation(exp_all[:, i:i + 1], exp_all[:, i:i + 1], Act.Exp)
            A_t = spool.tile([128, BL], f32, tag="A")
            nc.vector.tensor_scalar(
                out=A_t, in0=sel4_sb, scalar1=exp_all[:, i:i + 1],
                scalar2=maskp_sb[:, i:i + 1], op0=Alu.mult, op1=Alu.mult)
            o_t = opool.tile([128, N], f32)
            nc.sync.dma_start(out=o_t, in_=eo_ap[r0:r0 + 128, :])
            for c in range(NCH):
                nc.tensor.matmul(
                    content_ps[:, BL * c:BL * (c + 1)],
                    lhsT=o_t[:, 128 * c:128 * (c + 1)], rhs=A_t,
                    start=(i == 0), stop=(i == NT - 1))

        # ---- softmax denominator and final linear ----
        den_ps = psum.tile([BL, NT], f32, tag="aux")
        nc.tensor.matmul(den_ps, lhsT=sel4_sb, rhs=exp_all, start=True, stop=True)
        den4 = spool.tile([BL, 1], f32, tag="den")
        nc.vector.tensor_reduce(out=den4, in_=den_ps, axis=mybir.AxisListType.X,
                                op=Alu.add)
        inv4 = spool.tile([BL, 1], f32, tag="inv")
        nc.vector.reciprocal(inv4, den4)

        content_sb = const.tile([128, NCH * BL], f32)
        nc.vector.tensor_copy(content_sb, content_ps)

        fc_ps = psum.tile([BL, N], f32, tag="aux")    # content @ linT[:1024]
        fh_ps = psum.tile([BL, N], f32, tag="aux")    # hx      @ linT[1024:]
        for h in range(2):
            for c in range(NCH):
                nc.tensor.matmul(
                    fc_ps[:, 512 * h:512 * (h + 1)],
                    lhsT=content_sb[:, BL * c:BL * (c + 1)],
                    rhs=linT_sb[:, c, 512 * h:512 * (h + 1)],
                    start=(c == 0), stop=(c == NCH - 1))
                nc.tensor.matmul(
                    fh_ps[:, 512 * h:512 * (h + 1)],
                    lhsT=hxT_sb[:, c, :],
                    rhs=linT_sb[:, NCH + c, 512 * h:512 * (h + 1)],
                    start=(c == 0), stop=(c == NCH - 1))
        fin = const.tile([BL, N], f32)
        for h in range(2):
            nc.vector.scalar_tensor_tensor(
                out=fin[:, 512 * h:512 * (h + 1)],
                in0=fc_ps[:, 512 * h:512 * (h + 1)], scalar=inv4,
                in1=fh_ps[:, 512 * h:512 * (h + 1)],
                op0=Alu.mult, op1=Alu.add)
        fin2 = const.tile([BL, N], f32)
        nc.vector.tensor_add(fin2, fin, linb_sb)
        hx_sb = const.tile([BL, N], f32)
        nc.scalar.activation(hx_sb, fin2, Act.Tanh)
        nc.sync.dma_start(out=out[:], in_=hx_sb)

    nc.finalize()
    return nc


def kernel(**inputs):
    global LAST_RESULT
    from concourse.bass_utils import run_bass_kernel_spmd

    f = lambda k: np.asarray(inputs[k], dtype=np.float32)
    decoder_hx = f("decoder_hx")
    encoder_outputs = f("encoder_outputs")
    encoder_feature = f("encoder_feature")
    mask_tensor = f("mask_tensor")
    Ws_w, Ws_b = f("Ws_w"), f("Ws_b")
    v_w = f("v_w")
    lin_w, lin_b = f("lin_w"), f("lin_b")

    wsT = np.ascontiguousarray(Ws_w.T)
    linT = np.ascontiguousarray(lin_w.T)
    wsb = np.tile(Ws_b[None, :], (BL, 1))
    linb = np.tile(lin_b[None, :], (BL, 1))
    sel4 = np.tile(np.eye(BL, dtype=np.float32), (128 // BL, 1))
    rep4 = np.ascontiguousarray(sel4.T)
    ones128 = np.ones((1, 128), dtype=np.float32)

    in_maps = []
    for c in range(NCORES):
        b0 = BL * c
        in_maps.append({
            "ef": np.ascontiguousarray(
                encoder_feature[:, b0:b0 + BL, :]).reshape(ROWS, N),
            "eo": np.ascontiguousarray(
                encoder_outputs[:, b0:b0 + BL, :]).reshape(ROWS, N),
            "hxT": np.ascontiguousarray(decoder_hx[b0:b0 + BL, :].T),
            "wsT": wsT,
            "wsb": wsb,
            "vw": v_w.reshape(1, N),
            "maskp": np.ascontiguousarray(
                mask_tensor[b0:b0 + BL, :].reshape(BL, NT, T // NT // 1 // (T // NT) * (T // NT) // (T // NT) if False else NT // NT * (T // NT)).reshape(BL, NT, T // NT).transpose(2, 0, 1).reshape(128, NT))
            if False else np.ascontiguousarray(
                mask_tensor[b0:b0 + BL, :].reshape(BL, NT, T // NT).transpose(2, 0, 1).reshape(128, NT)),
            "linT": linT,
            "linb": linb,
            "sel4": sel4,
            "rep4": rep4,
            "ones128": ones128,
        })

    nc = _build()
    trace = bool(int(os.environ.get("BASS_KERNEL_TRACE", "0")))
    res = run_bass_kernel_spmd(nc, in_maps, list(range(NCORES)), trace=trace)
    LAST_RESULT = res
    return np.concatenate([res.results[c]["out"] for c in range(NCORES)], axis=0)


# revision 9
# speedup vs baseline: 288.5967x; 288.5967x over previous
"""Bahdanau additive attention on 8 Trainium2 NeuronCores.

Sharding: data-parallel over batch b (B=32 -> 4 per core); parameters replicated.

Per-core layout: encoder tensors (T=2048, bl=4, N=1024) flattened to rows=(t*4+b)
-> 64 row-tiles of [128, 1024].  Partition p of tile i holds (t = 32*i + p//4,
b = p%4).  Per tile:
  DVE : e_pre = f + dec_rep          (dec_rep[p,:] = dec_feature[p%4,:])
  ACT : e = tanh(e_pre)
  DVE : score_col = sum_n e * v_rep  (fused scalar_tensor_tensor accum)
  ACT : exp_col = exp(score_col)     (softmax max-shift unneeded: |scores| < 30)
  DVE : A = (sel4 * exp_col) * mask_col   [128, 4] one-hot-by-batch rhs
  PE  : content[nchunk, b] += o_tile[:, nchunk].T @ A   (8 matmuls, PSUM accum)
Tail: den[b] = sum_t exp (PE sel4 matmul + DVE reduce), content scaled by 1/den
inside the final linear; hx = tanh(lin_w @ [content; hx_dec] + lin_b) on PE/DVE/ACT.
v_b shifts all logits uniformly -> cancels in softmax -> dropped.
"""
import os
import sys

sys.path.insert(0, "/opt/trn_rl_repo")

import numpy as np

T, B, N = 2048, 32, 1024
NCORES = 8
BL = B // NCORES              # 4 local batches per core
ROWS = T * BL                 # 8192
NT = ROWS // 128              # 64 row tiles
NCH = N // 128                # 8 column chunks of N

LAST_RESULT = None            # BassKernelResults of the most recent run (for test.py)


def _build(reps=0):
    """reps=0: straight-line kernel (grading path). reps>0: wrap the whole body
    in a hardware For_i loop so bench.py can difference wall-clock over many
    on-device repetitions (no NTFF profiling available in this container)."""
    import concourse.bacc as bacc
    import concourse.tile as tile
    from concourse import mybir
    from contextlib import ExitStack

    dt = mybir.dt
    f32 = dt.float32
    Alu = mybir.AluOpType
    Act = mybir.ActivationFunctionType

    nc = bacc.Bacc("TRN2")
    ef = nc.dram_tensor("ef", [ROWS, N], f32, kind="ExternalInput")
    eo = nc.dram_tensor("eo", [ROWS, N], f32, kind="ExternalInput")
    hxT = nc.dram_tensor("hxT", [N, BL], f32, kind="ExternalInput")      # decoder_hx slice, transposed
    wsT = nc.dram_tensor("wsT", [N, N], f32, kind="ExternalInput")       # Ws_w.T (nin, nout)
    wsb = nc.dram_tensor("wsb", [BL, N], f32, kind="ExternalInput")      # Ws_b tiled 4x
    vw = nc.dram_tensor("vw", [1, N], f32, kind="ExternalInput")
    maskp = nc.dram_tensor("maskp", [128, NT], f32, kind="ExternalInput")
    linT = nc.dram_tensor("linT", [2 * N, N], f32, kind="ExternalInput") # lin_w.T (k, j)
    linb = nc.dram_tensor("linb", [BL, N], f32, kind="ExternalInput")    # lin_b tiled 4x
    sel4 = nc.dram_tensor("sel4", [128, BL], f32, kind="ExternalInput")  # tile(eye(4), (32,1))
    rep4 = nc.dram_tensor("rep4", [BL, 128], f32, kind="ExternalInput")  # sel4.T
    ones128 = nc.dram_tensor("ones128", [1, 128], f32, kind="ExternalInput")
    out = nc.dram_tensor("out", [BL, N], f32, kind="ExternalOutput")

    with tile.TileContext(nc) as tc, ExitStack() as ctx:
        if reps:
            ctx.enter_context(tc.For_i(0, reps, 1))
        const = ctx.enter_context(tc.tile_pool(name="const", bufs=1))
        wpool = ctx.enter_context(tc.tile_pool(name="weights", bufs=1))
        psum = ctx.enter_context(tc.tile_pool(name="psum", bufs=2, space="PSUM"))
        psc = ctx.enter_context(tc.tile_pool(name="psc", bufs=1, space="PSUM"))
        fpool = ctx.enter_context(tc.tile_pool(name="f", bufs=4))
        opool = ctx.enter_context(tc.tile_pool(name="o", bufs=4))
        e1pool = ctx.enter_context(tc.tile_pool(name="e1", bufs=2))
        e2pool = ctx.enter_context(tc.tile_pool(name="e2", bufs=2))
        spool = ctx.enter_context(tc.tile_pool(name="small", bufs=4))

        # ---- setup: params in ----
        wsT_sb = wpool.tile([128, NCH, N], f32)
        nc.sync.dma_start(out=wsT_sb, in_=wsT[:].rearrange("(c p) n -> p c n", p=128))
        linT_sb = wpool.tile([128, 2 * NCH, N], f32)
        nc.sync.dma_start(out=linT_sb, in_=linT[:].rearrange("(c p) j -> p c j", p=128))
        hxT_sb = const.tile([128, NCH, BL], f32)
        nc.sync.dma_start(out=hxT_sb, in_=hxT[:].rearrange("(c p) b -> p c b", p=128))
        wsb_sb = const.tile([BL, N], f32)
        nc.sync.dma_start(out=wsb_sb, in_=wsb[:])
        vw_sb = const.tile([1, N], f32)
        nc.sync.dma_start(out=vw_sb, in_=vw[:])
        maskp_sb = const.tile([128, NT], f32)
        nc.sync.dma_start(out=maskp_sb, in_=maskp[:])
        sel4_sb = const.tile([128, BL], f32)
        nc.sync.dma_start(out=sel4_sb, in_=sel4[:])
        rep4_sb = const.tile([BL, 128], f32)
        nc.sync.dma_start(out=rep4_sb, in_=rep4[:])
        ones_sb = const.tile([1, 128], f32)
        nc.sync.dma_start(out=ones_sb, in_=ones128[:])
        linb_sb = const.tile([BL, N], f32)
        nc.sync.dma_start(out=linb_sb, in_=linb[:])

        # ---- dec_feature = decoder_hx @ Ws_w.T + Ws_b, then replicate to 128 rows ----
        dec_ps = psum.tile([BL, N], f32, tag="aux")
        for h in range(2):
            for ci in range(NCH):
                nc.tensor.matmul(
                    dec_ps[:, 512 * h:512 * (h + 1)],
                    lhsT=hxT_sb[:, ci, :],
                    rhs=wsT_sb[:, ci, 512 * h:512 * (h + 1)],
                    start=(ci == 0), stop=(ci == NCH - 1))
        dec_sb = const.tile([BL, N], f32)
        nc.vector.tensor_add(dec_sb, dec_ps, wsb_sb)
        rep_ps = psum.tile([128, N], f32, tag="aux")
        for h in range(2):
            nc.tensor.matmul(rep_ps[:, 512 * h:512 * (h + 1)],
                             lhsT=rep4_sb, rhs=dec_sb[:, 512 * h:512 * (h + 1)],
                             start=True, stop=True)
        dec_rep = const.tile([128, N], f32)
        nc.vector.tensor_copy(dec_rep, rep_ps)

        # ---- v_w replicated to 128 rows via PE ----
        vrep_ps = psum.tile([128, N], f32, tag="aux")
        for h in range(2):
            nc.tensor.matmul(vrep_ps[:, 512 * h:512 * (h + 1)],
                             lhsT=ones_sb, rhs=vw_sb[:, 512 * h:512 * (h + 1)],
                             start=True, stop=True)
        v_rep = const.tile([128, N], f32)
        nc.vector.tensor_copy(v_rep, vrep_ps)

        exp_all = const.tile([128, NT], f32)
        content_ps = psc.tile([128, NCH * BL], f32)   # [n%128, chunk*4+b]

        ef_ap, eo_ap = ef[:], eo[:]
        # ---- main streaming loop ----
        for i in range(NT):
            r0 = 128 * i
            f_t = fpool.tile([128, N], f32)
            nc.sync.dma_start(out=f_t, in_=ef_ap[r0:r0 + 128, :])
            ep = e1pool.tile([128, N], f32)
            nc.vector.tensor_add(ep, f_t, dec_rep)
            e_t = e2pool.tile([128, N], f32)
            nc.scalar.activation(e_t, ep, Act.Tanh)
            dummy = spool.tile([128, 1], f32, tag="dummy")
            nc.vector.scalar_tensor_tensor(
                out=dummy.broadcast_to(e_t.shape), in0=e_t, scalar=1.0, in1=v_rep,
                op0=Alu.mult, op1=Alu.mult, accum_out=exp_all[:, i:i + 1])
            nc.scalar.activation(exp_all[:, i:i + 1], exp_all[:, i:i + 1], Act.Exp)
            A_t = spool.tile([128, BL], f32, tag="A")
            nc.vector.tensor_scalar(
                out=A_t, in0=sel4_sb, scalar1=exp_all[:, i:i + 1],
                scalar2=maskp_sb[:, i:i + 1], op0=Alu.mult, op1=Alu.mult)
            o_t = opool.tile([128, N], f32)
            nc.sync.dma_start(out=o_t, in_=eo_ap[r0:r0 + 128, :])
            for c in range(NCH):
                # NB: start=True clears has_written for the WHOLE psum bank, not
                # just the written region -> exactly one start for the bank.
                nc.tensor.matmul(
                    content_ps[:, BL * c:BL * (c + 1)],
                    lhsT=o_t[:, 128 * c:128 * (c + 1)], rhs=A_t,
                    start=(i == 0 and c == 0),
                    stop=(i == NT - 1 and c == NCH - 1),
                    skip_group_check=True)

        # ---- softmax denominator and final linear ----
        den_ps = psum.tile([BL, NT], f32, tag="aux")
        nc.tensor.matmul(den_ps, lhsT=sel4_sb, rhs=exp_all, start=True, stop=True)
        den4 = spool.tile([BL, 1], f32, tag="den")
        nc.vector.tensor_reduce(out=den4, in_=den_ps, axis=mybir.AxisListType.X,
                                op=Alu.add)
        inv4 = spool.tile([BL, 1], f32, tag="inv")
        nc.vector.reciprocal(inv4, den4)

        content_sb = const.tile([128, NCH * BL], f32)
        nc.vector.tensor_copy(content_sb, content_ps)

        fc_ps = psum.tile([BL, N], f32, tag="aux")    # content @ linT[:1024]
        fh_ps = psum.tile([BL, N], f32, tag="aux")    # hx      @ linT[1024:]
        for h in range(2):
            for c in range(NCH):
                nc.tensor.matmul(
                    fc_ps[:, 512 * h:512 * (h + 1)],
                    lhsT=content_sb[:, BL * c:BL * (c + 1)],
                    rhs=linT_sb[:, c, 512 * h:512 * (h + 1)],
                    start=(c == 0), stop=(c == NCH - 1))
                nc.tensor.matmul(
                    fh_ps[:, 512 * h:512 * (h + 1)],
                    lhsT=hxT_sb[:, c, :],
                    rhs=linT_sb[:, NCH + c, 512 * h:512 * (h + 1)],
                    start=(c == 0), stop=(c == NCH - 1))
        fh_sb = const.tile([BL, N], f32)
        nc.vector.tensor_add(fh_sb, fh_ps, linb_sb)   # one PSUM operand max per DVE op
        fin = const.tile([BL, N], f32)
        for h in range(2):
            nc.vector.scalar_tensor_tensor(
                out=fin[:, 512 * h:512 * (h + 1)],
                in0=fc_ps[:, 512 * h:512 * (h + 1)], scalar=inv4,
                in1=fh_sb[:, 512 * h:512 * (h + 1)],
                op0=Alu.mult, op1=Alu.add)
        hx_sb = const.tile([BL, N], f32)
        nc.scalar.activation(hx_sb, fin, Act.Tanh)
        nc.sync.dma_start(out=out[:], in_=hx_sb)

    nc.finalize()
    return nc


def _make_in_maps(inputs):
    f = lambda k: np.asarray(inputs[k], dtype=np.float32)
    decoder_hx = f("decoder_hx")
    encoder_outputs = f("encoder_outputs")
    encoder_feature = f("encoder_feature")
    mask_tensor = f("mask_tensor")
    Ws_w, Ws_b = f("Ws_w"), f("Ws_b")
    v_w = f("v_w")
    lin_w, lin_b = f("lin_w"), f("lin_b")

    wsT = np.ascontiguousarray(Ws_w.T)
    linT = np.ascontiguousarray(lin_w.T)
    wsb = np.tile(Ws_b[None, :], (BL, 1))
    linb = np.tile(lin_b[None, :], (BL, 1))
    sel4 = np.tile(np.eye(BL, dtype=np.float32), (128 // BL, 1))
    rep4 = np.ascontiguousarray(sel4.T)
    ones128 = np.ones((1, 128), dtype=np.float32)

    in_maps = []
    for c in range(NCORES):
        b0 = BL * c
        in_maps.append({
            "ef": np.ascontiguousarray(
                encoder_feature[:, b0:b0 + BL, :]).reshape(ROWS, N),
            "eo": np.ascontiguousarray(
                encoder_outputs[:, b0:b0 + BL, :]).reshape(ROWS, N),
            "hxT": np.ascontiguousarray(decoder_hx[b0:b0 + BL, :].T),
            "wsT": wsT,
            "wsb": wsb,
            "vw": v_w.reshape(1, N),
            "maskp": np.ascontiguousarray(
                mask_tensor[b0:b0 + BL, :].reshape(BL, NT, T // NT)
                .transpose(2, 0, 1).reshape(128, NT)),
            "linT": linT,
            "linb": linb,
            "sel4": sel4,
            "rep4": rep4,
            "ones128": ones128,
        })
    return in_maps


def kernel(**inputs):
    global LAST_RESULT
    from concourse.bass_utils import run_bass_kernel_spmd

    in_maps = _make_in_maps(inputs)
    nc = _build()
    res = run_bass_kernel_spmd(nc, in_maps, list(range(NCORES)))
    LAST_RESULT = res
    return np.concatenate([res.results[c]["out"] for c in range(NCORES)], axis=0)
